# revision 3
# baseline (speedup 1.0000x reference)
"""Trainium2 Bass kernel for ChannelAttentionModule.

Reference computation (per batch item b):
    avg[b, c] = mean(x[b, c, :, :]);  mx[b, c] = max(x[b, c, :, :])
    out[b] = sigmoid(MLP(avg[b]) + MLP(mx[b]))  with MLP(v) = w2 @ relu(w1 @ v)
    output shape [B, C, 1, 1]

Strategy (8 NeuronCores, data-parallel over batch):
  - Each core gets 2 batch items: x shard [2, 256, 128, 128] -> viewed as
    [512, 16384] (row = b*256 + channel, channels land on SBUF partitions).
  - Stream spatial chunks [128, CHUNK]; ScalarE (ACT) computes per-chunk sums
    via activation(Copy, accum_out=...), VectorE (DVE) computes per-chunk
    maxes via reduce_max.  The two engines each make one pass so both stay
    under the ~93us/core HBM roofline (33.5 MB @ ~360 GB/s).
  - Tiny 2-layer MLP (256x256 weights, transposed on host) runs on the PE
    over rhs [128, 4] = [avg_b0, avg_b1, max_b0, max_b1] per K-tile, then
    sigmoid(avgout+maxout) and a [256, 2] store per core.
"""

import numpy as np

B, C, H, W = 16, 256, 128, 128
NCORES = 8
BLOC = B // NCORES            # batch items per core
HWSP = H * W                  # spatial size per channel
CHUNK = 4096                  # spatial elements per streamed tile
NCHUNK = HWSP // CHUNK        # chunks per (batch, channel-tile) group
CT = C // 128                 # channel tiles per batch item

_CACHE = {}


def _build_module():
    from contextlib import ExitStack

    import concourse.bacc as bacc
    import concourse.mybir as mybir
    import concourse.tile as tile

    f32 = mybir.dt.float32
    AF = mybir.ActivationFunctionType
    AX = mybir.AxisListType
    ALU = mybir.AluOpType

    nc = bacc.Bacc(
        "TRN2",
        target_bir_lowering=False,
        debug=False,
        enable_asserts=False,
        num_devices=NCORES,
    )
    x = nc.dram_tensor("x", [BLOC * C, HWSP], f32, kind="ExternalInput").ap()
    w1t = nc.dram_tensor("w1t", [C, C], f32, kind="ExternalInput").ap()
    w2t = nc.dram_tensor("w2t", [C, C], f32, kind="ExternalInput").ap()
    outT = nc.dram_tensor("outT", [C, BLOC], f32, kind="ExternalOutput").ap()

    NGROUP = BLOC * CT        # (b, ct) groups
    NP = NGROUP * NCHUNK      # total partial-reduction slots

    with tile.TileContext(nc) as tc:
        with ExitStack() as ctx:
            xpool = ctx.enter_context(tc.tile_pool(name="xpool", bufs=8))
            spool = ctx.enter_context(tc.tile_pool(name="spool", bufs=1))
            psum = ctx.enter_context(tc.tile_pool(name="psum", bufs=1, space="PSUM"))

            # Weights: w1s[:, kt*C + o] = w1t[kt*128 + k, o] (lhsT layout)
            w1s = spool.tile([128, 2 * C], f32)
            w2s = spool.tile([128, 2 * C], f32)
            for kt in range(2):
                nc.sync.dma_start(w1s[:, kt * C:(kt + 1) * C], w1t[kt * 128:(kt + 1) * 128, :])
                nc.sync.dma_start(w2s[:, kt * C:(kt + 1) * C], w2t[kt * 128:(kt + 1) * 128, :])

            sum_parts = spool.tile([128, NP], f32)
            max_parts = spool.tile([128, NP], f32)
            scratch = spool.tile([128, CHUNK], f32)
            dummy = spool.tile([128, NCHUNK], f32)

            # rhs tiles for the MLP: per K-tile ct, cols = [avg_b0, avg_b1, max_b0, max_b1]
            vts = [spool.tile([128, 2 * BLOC], f32, name=f"v{ct}") for ct in range(CT)]

            groups = [(b, ct) for b in range(BLOC) for ct in range(CT)]

            # Main streaming pass: DMA in, ACT sums, DVE maxes.
            for g, (b, ct) in enumerate(groups):
                row0 = b * C + ct * 128
                for j in range(NCHUNK):
                    col = g * NCHUNK + j
                    xt = xpool.tile([128, CHUNK], f32, tag="x", name="xt")
                    nc.sync.dma_start(xt[:], x[row0:row0 + 128, j * CHUNK:(j + 1) * CHUNK])
                    nc.scalar.activation(
                        scratch[:], xt[:], AF.Copy,
                        accum_out=sum_parts[:, col:col + 1],
                    )
                    nc.vector.reduce_max(max_parts[:, col:col + 1], xt[:], axis=AX.X)

            # Combine partials into the MLP rhs tiles.
            for g, (b, ct) in enumerate(groups):
                c0 = g * NCHUNK
                # avg: sum partials * (1/HW) then accum-add -> v[:, b]
                nc.vector.tensor_scalar(
                    dummy[:], sum_parts[:, c0:c0 + NCHUNK], 1.0 / HWSP, None,
                    ALU.mult, ALU.add, accum_out=vts[ct][:, b:b + 1],
                )
                nc.vector.reduce_max(
                    vts[ct][:, BLOC + b:BLOC + b + 1], max_parts[:, c0:c0 + NCHUNK], axis=AX.X,
                )

            # MLP layer 1 + ReLU
            hs = [spool.tile([128, 2 * BLOC], f32, name=f"h{ot}") for ot in range(CT)]
            for ot in range(CT):
                ph = psum.tile([128, 2 * BLOC], f32, name=f"ph{ot}")
                for kt in range(CT):
                    nc.tensor.matmul(
                        ph[:],
                        w1s[:, kt * C + ot * 128: kt * C + (ot + 1) * 128],
                        vts[kt][:],
                        start=(kt == 0), stop=(kt == CT - 1),
                    )
                nc.scalar.activation(hs[ot][:], ph[:], AF.Relu)

            # MLP layer 2 + add paths + sigmoid + store
            for ot in range(CT):
                py = psum.tile([128, 2 * BLOC], f32, name=f"py{ot}")
                for kt in range(CT):
                    nc.tensor.matmul(
                        py[:],
                        w2s[:, kt * C + ot * 128: kt * C + (ot + 1) * 128],
                        hs[kt][:],
                        start=(kt == 0), stop=(kt == CT - 1),
                    )
                ysb = spool.tile([128, 2 * BLOC], f32, name=f"y{ot}")
                nc.vector.tensor_copy(ysb[:], py[:])
                zz = spool.tile([128, BLOC], f32, name=f"z{ot}")
                nc.vector.tensor_add(zz[:], ysb[:, 0:BLOC], ysb[:, BLOC:2 * BLOC])
                osb = spool.tile([128, BLOC], f32, name=f"o{ot}")
                nc.scalar.activation(osb[:], zz[:], AF.Sigmoid)
                nc.sync.dma_start(outT[ot * 128:(ot + 1) * 128, :], osb[:])

    nc.compile()
    return nc


def _get_module():
    if "nc" not in _CACHE:
        _CACHE["nc"] = _build_module()
    return _CACHE["nc"]


def _run(inputs, trace=False):
    from concourse.bass_utils import run_bass_kernel_spmd

    nc = _get_module()
    x = np.ascontiguousarray(np.asarray(inputs["x"], dtype=np.float32))
    w1t = np.ascontiguousarray(np.asarray(inputs["w1"], dtype=np.float32).T)
    w2t = np.ascontiguousarray(np.asarray(inputs["w2"], dtype=np.float32).T)

    in_maps = []
    for c in range(NCORES):
        xs = x[c * BLOC:(c + 1) * BLOC].reshape(BLOC * C, HWSP)
        in_maps.append({"x": np.ascontiguousarray(xs), "w1t": w1t, "w2t": w2t})

    res = run_bass_kernel_spmd(
        nc, in_maps, core_ids=list(range(NCORES)),
        trace=trace, trace_cores=[0] if trace else None,
    )
    out = np.empty((B, C), dtype=np.float32)
    for c in range(NCORES):
        out[c * BLOC:(c + 1) * BLOC] = res.results[c]["outT"].T
    return out.reshape(B, C, 1, 1), res.exec_time_ns


def kernel(**inputs):
    out, _ = _run(inputs, trace=False)
    return out
